# revision 1
# baseline (speedup 1.0000x reference)
"""Trainium2 Bass kernel for nn_ExtractPatchesPositionLayer.

Reference semantics: per image b, bilinear-translate the (522,522,1) padded
object by t = -positions[b] (tfa.translate: out(y,x) = img(y+py, x+px),
zero fill outside), then center-crop 5px -> (512,512,1).

Because the shift is constant per image, floor/frac of the offset give an
integer window start (A,B) into the (zero-margin-padded) image plus four
constant bilinear corner weights. The whole bilinear then collapses into two
accumulating PE matmuls per 127-row chunk:

    psum[m, j] = sum_k Bv0[k, m] * W[k, j] + sum_k Bv1[k, m] * W[k, j+1]

with banded 128x127 matrices
    Bv0 = c00*I + c10*S,  Bv1 = c01*I + c11*S
    (I[k,m] = d_{k,m}, S[k,m] = d_{k,m+1};
     c00=(1-wy)(1-wx), c10=wy(1-wx), c01=(1-wy)wx, c11=wy*wx)

The per-image window is fetched with dynamic HWDGE DMAs: host-precomputed
flat element offsets (int32 data) are reg_load-ed into a small pool of
rotating SP registers and used as runtime AP offsets, so one SPMD program
serves all cores with no data-dependent immediates. (Indirect/gather DMA was
tried first but SWDGE lands every gather descriptor on DMA engine 0 —
1.4 ms serialized; dynamic HWDGE DMAs split across all 16 engines.)
Sharding: batch 256 -> 32 images x 8 cores, embarrassingly parallel,
no communication.
"""

from dataclasses import dataclass

import numpy as np

import concourse.bacc as bacc
import concourse.bass as bass
import concourse.mybir as mybir
import concourse.tile as tile
from concourse.bass_utils import run_bass_kernel_spmd


@dataclass(frozen=True)
class Cfg:
    bpc: int      # images per core
    n: int        # output height/width
    wpad: int     # padded input height/width (with zero margin)
    chunk: int    # output rows per matmul chunk (<=127)

    @property
    def win(self):  # window width loaded per chunk
        return self.n + 1

    @property
    def chunks(self):
        out = []
        r = 0
        while r < self.n:
            nr = min(self.chunk, self.n - r)
            out.append((r, nr))
            r += nr
        return out

    @property
    def nbig(self):
        return sum(1 for _, nr in self.chunks if nr == self.chunk)

    @property
    def rem(self):  # (row0, nrows) of the non-uniform trailing chunk, if any
        r = self.chunks[self.nbig:]
        assert len(r) <= 1
        return r[0] if r else None


def build_nc(cfg: Cfg) -> bass.Bass:
    BPC, N, WPAD, WIN = cfg.bpc, cfg.n, cfg.wpad, cfg.win
    CH = cfg.chunk
    nbig = cfg.nbig
    rem = cfg.rem
    P = CH + 1
    PS = (rem[1] + 1) if rem else 1  # partitions of the remainder gather
    TOT = BPC * WPAD * WPAD
    f32 = mybir.dt.float32
    i32 = mybir.dt.int32

    nc = bacc.Bacc("TRN2", target_bir_lowering=False, debug=False)
    x_d = nc.declare_dram_parameter("x", [BPC, WPAD, WPAD], f32, isOutput=False)
    offs_d = nc.declare_dram_parameter("offs", [1, BPC * 2], i32, isOutput=False)
    wmat_d = nc.declare_dram_parameter("wmat", [BPC, 128, 4], f32, isOutput=False)
    dmat_d = nc.declare_dram_parameter("dmat", [128, 2 * CH], f32, isOutput=False)
    y_d = nc.declare_dram_parameter("y", [BPC, N, N], f32, isOutput=True)

    with tile.TileContext(nc) as tc:
        with (
            tc.tile_pool(name="const", bufs=1) as constp,
            tc.tile_pool(name="bmat", bufs=3) as bmatp,
            tc.tile_pool(name="win", bufs=3) as winp,
            tc.tile_pool(name="outp", bufs=3) as outp,
            tc.tile_pool(name="ps", bufs=6, space="PSUM") as psp,
        ):
            dmat_sb = constp.tile([128, 2 * CH], f32, tag="dmat")
            nc.sync.dma_start(dmat_sb[:], dmat_d[:, :])
            wmat_sb = constp.tile([128, BPC * 4], f32, tag="wmat")
            nc.sync.dma_start(
                wmat_sb[:].rearrange("p (i q) -> p i q", q=4),
                wmat_d[:, :, :].transpose([1, 0, 2]),
            )
            offs_sb = constp.tile([1, BPC * 2], i32, tag="offs")
            nc.sync.dma_start(offs_sb[:], offs_d[:, :])
            d0 = dmat_sb[:, 0:CH]
            d1 = dmat_sb[:, CH:2 * CH]

            # two register pools, one per HWDGE ring (SP + ACT); alternating
            # the big window loads across both rings doubles descriptor-gen
            # fan-out (a single dynamic DMA's descriptors serialize on one
            # DMA engine otherwise)
            off_max = TOT - 1
            pools = []
            for eng_t, eng in ((mybir.EngineType.SP, nc.sync),
                               (mybir.EngineType.Activation, nc.scalar)):
                regs = [nc.alloc_register(eng_t, f"dynoff_{eng_t}_{k}")
                        for k in range(min(8, 2 * BPC))]
                svs = [nc.snap(r, donate=True, min_val=0, max_val=off_max)
                       for r in regs]
                pools.append((eng, regs, svs))

            for i in range(BPC):
                # per-image banded matrices Bv0, Bv1 on DVE
                b0 = bmatp.tile([128, CH], f32, tag="b0")
                b1 = bmatp.tile([128, CH], f32, tag="b1")
                t0 = bmatp.tile([128, CH], f32, tag="t0")
                t1 = bmatp.tile([128, CH], f32, tag="t1")
                c00 = wmat_sb[:, 4 * i + 0: 4 * i + 1]
                c10 = wmat_sb[:, 4 * i + 1: 4 * i + 2]
                c01 = wmat_sb[:, 4 * i + 2: 4 * i + 3]
                c11 = wmat_sb[:, 4 * i + 3: 4 * i + 4]
                nc.scalar.mul(t0[:], d1, c10)
                nc.scalar.mul(b0[:], d0, c00)
                nc.vector.tensor_add(b0[:], b0[:], t0[:])
                nc.scalar.mul(t1[:], d1, c11)
                nc.scalar.mul(b1[:], d0, c01)
                nc.vector.tensor_add(b1[:], b1[:], t1[:])

                # dynamic flat element offsets, host-precomputed per DMA;
                # one strided DMA loads all uniform chunks:
                # wt_big[p, c, w] = x.flat[off_big + (c*CH + p)*WPAD + w]
                eng, regs, svs = pools[i % 2]
                nreg = len(regs)
                kb = (2 * i) % nreg
                eng.reg_load(regs[kb], offs_sb[0:1, 2 * i: 2 * i + 1])
                wt_big = winp.tile([P, nbig * WIN], f32, tag="wt_big")
                eng.dma_start(
                    wt_big[:].rearrange("p (c w) -> p c w", w=WIN),
                    bass.AP(x_d, svs[kb],
                            [[WPAD, P], [CH * WPAD, nbig], [1, WIN]]),
                )
                if rem:
                    ks = (2 * i + 1) % nreg
                    eng.reg_load(regs[ks], offs_sb[0:1, 2 * i + 1: 2 * i + 2])
                    wt_s = winp.tile([PS, WIN], f32, tag="wt_s")
                    eng.dma_start(
                        wt_s[:],
                        bass.AP(x_d, svs[ks], [[WPAD, PS], [1, WIN]]),
                    )

                ob_big = outp.tile([CH, nbig * N], f32, tag="ob_big")
                for c in range(nbig):
                    ps = psp.tile([CH, N], f32, tag="ps")
                    rhs0 = wt_big[:P, c * WIN: c * WIN + N]
                    rhs1 = wt_big[:P, c * WIN + 1: c * WIN + 1 + N]
                    nc.tensor.matmul(out=ps[:], lhsT=b0[:P, :], rhs=rhs0,
                                     start=True, stop=False)
                    nc.tensor.matmul(out=ps[:], lhsT=b1[:P, :], rhs=rhs1,
                                     start=False, stop=True)
                    nc.scalar.copy(ob_big[:, c * N:(c + 1) * N], ps[:])
                # store the uniform chunks with one strided DMA:
                # y[i, c*CH + m, j] = ob_big[m, c*N + j]
                nc.sync.dma_start(
                    bass.AP(y_d, i * (N * N),
                            [[N, CH], [CH * N, nbig], [1, N]]),
                    ob_big[:].rearrange("p (c w) -> p c w", w=N),
                )
                if rem:
                    r0r, nrr = rem
                    ps_s = psp.tile([CH, N], f32, tag="ps")
                    ob_s = outp.tile([max(nrr, 1), N], f32, tag="ob_s")
                    nc.tensor.matmul(out=ps_s[:nrr, :],
                                     lhsT=b0[:nrr + 1, :nrr],
                                     rhs=wt_s[:nrr + 1, 0:N],
                                     start=True, stop=False)
                    nc.tensor.matmul(out=ps_s[:nrr, :],
                                     lhsT=b1[:nrr + 1, :nrr],
                                     rhs=wt_s[:nrr + 1, 1:N + 1],
                                     start=False, stop=True)
                    nc.scalar.copy(ob_s[:nrr, :], ps_s[:nrr, :])
                    nc.sync.dma_start(y_d[i, r0r:r0r + nrr, :], ob_s[:nrr, :])
    nc.compile()
    return nc


def host_prep(padded: np.ndarray, positions: np.ndarray, n_cores: int, chunk: int):
    """Shard + build metadata. padded: (B, npad, npad) f32, positions: (B, 2)."""
    B, npad, _ = padded.shape
    n = npad - 10
    bpc = B // n_cores
    win = n + 1

    px = positions[:, 0].astype(np.float32)
    py = positions[:, 1].astype(np.float32)
    fy = np.floor(py)
    fx = np.floor(px)
    ay = (5 + fy).astype(np.int64)
    ax = (5 + fx).astype(np.int64)
    wy = (py - fy).astype(np.float32)
    wx = (px - fx).astype(np.float32)

    m_lo = int(max(0, -min(ay.min(), ax.min())))
    m_hi = int(max(0, max(ay.max(), ax.max()) + win - npad))
    wpad = npad + m_lo + m_hi

    pp = np.zeros((B, wpad, wpad), dtype=np.float32)
    pp[:, m_lo:m_lo + npad, m_lo:m_lo + npad] = padded

    c00 = ((1 - wy) * (1 - wx)).astype(np.float32)
    c10 = (wy * (1 - wx)).astype(np.float32)
    c01 = ((1 - wy) * wx).astype(np.float32)
    c11 = (wy * wx).astype(np.float32)

    dmat = np.zeros((128, 2 * chunk), dtype=np.float32)
    for m in range(chunk):
        dmat[m, m] = 1.0            # I
        dmat[m + 1, chunk + m] = 1.0  # S

    cfg = Cfg(bpc=bpc, n=n, wpad=wpad, chunk=chunk)
    nbig = cfg.nbig
    rem = cfg.rem
    P = chunk + 1
    PS = (rem[1] + 1) if rem else 1

    in_maps = []
    for cidx in range(n_cores):
        sl = slice(cidx * bpc, (cidx + 1) * bpc)
        A = (ay[sl] + m_lo).astype(np.int64)
        Bc = (ax[sl] + m_lo).astype(np.int64)
        base = np.arange(bpc, dtype=np.int64) * (wpad * wpad)
        # flat element offsets: big windowed DMA start, remainder-chunk start
        off_big = base + A * wpad + Bc
        if rem:
            off_small = off_big + (rem[0]) * wpad
        else:
            off_small = np.zeros_like(off_big)
        offs = np.empty((1, bpc * 2), dtype=np.int32)
        offs[0, 0::2] = off_big
        offs[0, 1::2] = off_small
        wmat = np.empty((bpc, 128, 4), dtype=np.float32)
        wmat[:, :, 0] = c00[sl][:, None]
        wmat[:, :, 1] = c10[sl][:, None]
        wmat[:, :, 2] = c01[sl][:, None]
        wmat[:, :, 3] = c11[sl][:, None]
        in_maps.append({
            "x": np.ascontiguousarray(pp[sl]),
            "offs": offs,
            "wmat": wmat,
            "dmat": dmat,
        })
    return cfg, in_maps


N_CORES = 8
CHUNK = 127
_nc_cache: dict = {}


def kernel(padded_obj: np.ndarray, positions: np.ndarray) -> np.ndarray:
    padded_obj = np.asarray(padded_obj)
    positions = np.asarray(positions)
    B, npad, _, C = padded_obj.shape
    cfg, in_maps = host_prep(
        padded_obj.reshape(B, npad, npad).astype(np.float32, copy=False),
        positions, N_CORES, CHUNK)

    nc = _nc_cache.get(cfg)
    if nc is None:
        nc = build_nc(cfg)
        _nc_cache[cfg] = nc

    res = run_bass_kernel_spmd(nc, in_maps, core_ids=list(range(N_CORES)))
    out = np.concatenate([r["y"] for r in res.results], axis=0)
    return out.reshape(B, cfg.n, cfg.n, 1).astype(np.float32, copy=False)



# revision 3
# speedup vs baseline: 1.3394x; 1.3394x over previous
"""Trainium2 Bass kernel for nn_ExtractPatchesPositionLayer.

Reference semantics: per image b, bilinear-translate the (522,522,1) padded
object by t = -positions[b] (tfa.translate: out(y,x) = img(y+py, x+px),
zero fill outside), then center-crop 5px -> (512,512,1).

Because the shift is constant per image, floor/frac of the offset give an
integer window start (A,B) into the (zero-margin-padded) image plus four
constant bilinear corner weights. The whole bilinear then collapses into two
accumulating PE matmuls per 127-row chunk:

    psum[m, j] = sum_k Bv0[k, m] * W[k, j] + sum_k Bv1[k, m] * W[k, j+1]

with banded 128x127 matrices
    Bv0 = c00*I + c10*S,  Bv1 = c01*I + c11*S
    (I[k,m] = d_{k,m}, S[k,m] = d_{k,m+1};
     c00=(1-wy)(1-wx), c10=wy(1-wx), c01=(1-wy)wx, c11=wy*wx)

The per-image window is fetched with dynamic HWDGE DMAs: host-precomputed
flat element offsets (int32 data) are reg_load-ed into a small pool of
rotating SP registers and used as runtime AP offsets, so one SPMD program
serves all cores with no data-dependent immediates. (Indirect/gather DMA was
tried first but SWDGE lands every gather descriptor on DMA engine 0 —
1.4 ms serialized; dynamic HWDGE DMAs split across all 16 engines.)
Sharding: batch 256 -> 32 images x 8 cores, embarrassingly parallel,
no communication.
"""

from dataclasses import dataclass

import numpy as np

import concourse.bacc as bacc
import concourse.bass as bass
import concourse.mybir as mybir
import concourse.tile as tile
from concourse.bass_utils import run_bass_kernel_spmd


@dataclass(frozen=True)
class Cfg:
    bpc: int      # images per core
    n: int        # output height/width
    wpad: int     # padded input height/width (with zero margin)
    chunk: int    # output rows per matmul chunk (<=127)

    @property
    def win(self):  # window width loaded per chunk
        return self.n + 1

    @property
    def chunks(self):
        out = []
        r = 0
        while r < self.n:
            nr = min(self.chunk, self.n - r)
            out.append((r, nr))
            r += nr
        return out

    @property
    def nbig(self):
        return sum(1 for _, nr in self.chunks if nr == self.chunk)

    @property
    def rem(self):  # (row0, nrows) of the non-uniform trailing chunk, if any
        r = self.chunks[self.nbig:]
        assert len(r) <= 1
        return r[0] if r else None


def build_nc(cfg: Cfg) -> bass.Bass:
    BPC, N, WPAD, WIN = cfg.bpc, cfg.n, cfg.wpad, cfg.win
    CH = cfg.chunk
    nbig = cfg.nbig
    rem = cfg.rem
    P = CH + 1
    PS = (rem[1] + 1) if rem else 1  # partitions of the remainder gather
    TOT = BPC * WPAD * WPAD
    f32 = mybir.dt.float32
    i32 = mybir.dt.int32

    nc = bacc.Bacc("TRN2", target_bir_lowering=False, debug=False)
    x_d = nc.declare_dram_parameter("x", [BPC, WPAD, WPAD], f32, isOutput=False)
    offs_d = nc.declare_dram_parameter("offs", [1, BPC * 2], i32, isOutput=False)
    wmat_d = nc.declare_dram_parameter("wmat", [BPC, 128, 4], f32, isOutput=False)
    dmat_d = nc.declare_dram_parameter("dmat", [128, 2 * CH], f32, isOutput=False)
    y_d = nc.declare_dram_parameter("y", [BPC, N, N], f32, isOutput=True)

    with tile.TileContext(nc) as tc:
        with (
            tc.tile_pool(name="const", bufs=1) as constp,
            tc.tile_pool(name="bmat", bufs=3) as bmatp,
            tc.tile_pool(name="win", bufs=3) as winp,
            tc.tile_pool(name="outp", bufs=3) as outp,
            tc.tile_pool(name="ps", bufs=6, space="PSUM") as psp,
        ):
            dmat_sb = constp.tile([128, 2 * CH], f32, tag="dmat")
            nc.sync.dma_start(dmat_sb[:], dmat_d[:, :])
            wmat_sb = constp.tile([128, BPC * 4], f32, tag="wmat")
            nc.sync.dma_start(
                wmat_sb[:].rearrange("p (i q) -> p i q", q=4),
                wmat_d[:, :, :].transpose([1, 0, 2]),
            )
            offs_sb = constp.tile([1, BPC * 2], i32, tag="offs")
            nc.sync.dma_start(offs_sb[:], offs_d[:, :])
            d0 = dmat_sb[:, 0:CH]
            d1 = dmat_sb[:, CH:2 * CH]

            # two register pools, one per HWDGE ring (SP + ACT); alternating
            # the big window loads across both rings doubles descriptor-gen
            # fan-out (a single dynamic DMA's descriptors serialize on one
            # DMA engine otherwise)
            off_max = TOT - 1
            pools = []
            for eng_t, eng in ((mybir.EngineType.SP, nc.sync),
                               (mybir.EngineType.Activation, nc.scalar)):
                regs = [nc.alloc_register(eng_t, f"dynoff_{eng_t}_{k}")
                        for k in range(min(8, 2 * BPC))]
                svs = [nc.snap(r, donate=True, min_val=0, max_val=off_max)
                       for r in regs]
                pools.append((eng, regs, svs))

            for i in range(BPC):
                # per-image banded matrices Bv0, Bv1 on DVE
                b0 = bmatp.tile([128, CH], f32, tag="b0")
                b1 = bmatp.tile([128, CH], f32, tag="b1")
                t0 = bmatp.tile([128, CH], f32, tag="t0")
                t1 = bmatp.tile([128, CH], f32, tag="t1")
                c00 = wmat_sb[:, 4 * i + 0: 4 * i + 1]
                c10 = wmat_sb[:, 4 * i + 1: 4 * i + 2]
                c01 = wmat_sb[:, 4 * i + 2: 4 * i + 3]
                c11 = wmat_sb[:, 4 * i + 3: 4 * i + 4]
                nc.scalar.mul(t0[:], d1, c10)
                nc.scalar.mul(b0[:], d0, c00)
                nc.vector.tensor_add(b0[:], b0[:], t0[:])
                nc.scalar.mul(t1[:], d1, c11)
                nc.scalar.mul(b1[:], d0, c01)
                nc.vector.tensor_add(b1[:], b1[:], t1[:])

                # dynamic flat element offsets, host-precomputed per DMA;
                # one strided DMA loads all uniform chunks:
                # wt_big[p, c, w] = x.flat[off_big + (c*CH + p)*WPAD + w]
                eng, regs, svs = pools[i % 2]
                nreg = len(regs)
                kb = (2 * i) % nreg
                eng.reg_load(regs[kb], offs_sb[0:1, 2 * i: 2 * i + 1])
                wt_big = winp.tile([P, nbig * WIN], f32, tag="wt_big")
                eng.dma_start(
                    wt_big[:].rearrange("p (c w) -> p c w", w=WIN),
                    bass.AP(x_d, svs[kb],
                            [[WPAD, P], [CH * WPAD, nbig], [1, WIN]]),
                )
                if rem:
                    ks = (2 * i + 1) % nreg
                    eng.reg_load(regs[ks], offs_sb[0:1, 2 * i + 1: 2 * i + 2])
                    wt_s = winp.tile([PS, WIN], f32, tag="wt_s")
                    eng.dma_start(
                        wt_s[:],
                        bass.AP(x_d, svs[ks], [[WPAD, PS], [1, WIN]]),
                    )

                ob_big = outp.tile([CH, nbig * N], f32, tag="ob_big")
                for c in range(nbig):
                    ps = psp.tile([CH, N], f32, tag="ps")
                    rhs0 = wt_big[:P, c * WIN: c * WIN + N]
                    rhs1 = wt_big[:P, c * WIN + 1: c * WIN + 1 + N]
                    nc.tensor.matmul(out=ps[:], lhsT=b0[:P, :], rhs=rhs0,
                                     start=True, stop=False)
                    nc.tensor.matmul(out=ps[:], lhsT=b1[:P, :], rhs=rhs1,
                                     start=False, stop=True)
                    nc.scalar.copy(ob_big[:, c * N:(c + 1) * N], ps[:])
                # store the uniform chunks with one strided DMA. SWDGE
                # (gpsimd): HWDGE sends every SBUF->HBM descriptor to SDMA
                # engine 0 (trace: eng64 = 1.4ms busy, the critical path);
                # SWDGE's CounterMachine spreads them over all 16 engines.
                nc.gpsimd.dma_start(
                    bass.AP(y_d, i * (N * N),
                            [[N, CH], [CH * N, nbig], [1, N]]),
                    ob_big[:].rearrange("p (c w) -> p c w", w=N),
                )
                if rem:
                    r0r, nrr = rem
                    ps_s = psp.tile([CH, N], f32, tag="ps")
                    ob_s = outp.tile([max(nrr, 1), N], f32, tag="ob_s")
                    nc.tensor.matmul(out=ps_s[:nrr, :],
                                     lhsT=b0[:nrr + 1, :nrr],
                                     rhs=wt_s[:nrr + 1, 0:N],
                                     start=True, stop=False)
                    nc.tensor.matmul(out=ps_s[:nrr, :],
                                     lhsT=b1[:nrr + 1, :nrr],
                                     rhs=wt_s[:nrr + 1, 1:N + 1],
                                     start=False, stop=True)
                    nc.scalar.copy(ob_s[:nrr, :], ps_s[:nrr, :])
                    nc.gpsimd.dma_start(y_d[i, r0r:r0r + nrr, :], ob_s[:nrr, :])
    nc.compile()
    return nc


def host_prep(padded: np.ndarray, positions: np.ndarray, n_cores: int, chunk: int):
    """Shard + build metadata. padded: (B, npad, npad) f32, positions: (B, 2)."""
    B, npad, _ = padded.shape
    n = npad - 10
    bpc = B // n_cores
    win = n + 1

    px = positions[:, 0].astype(np.float32)
    py = positions[:, 1].astype(np.float32)
    fy = np.floor(py)
    fx = np.floor(px)
    ay = (5 + fy).astype(np.int64)
    ax = (5 + fx).astype(np.int64)
    wy = (py - fy).astype(np.float32)
    wx = (px - fx).astype(np.float32)

    m_lo = int(max(0, -min(ay.min(), ax.min())))
    m_hi = int(max(0, max(ay.max(), ax.max()) + win - npad))
    wpad = npad + m_lo + m_hi

    pp = np.zeros((B, wpad, wpad), dtype=np.float32)
    pp[:, m_lo:m_lo + npad, m_lo:m_lo + npad] = padded

    c00 = ((1 - wy) * (1 - wx)).astype(np.float32)
    c10 = (wy * (1 - wx)).astype(np.float32)
    c01 = ((1 - wy) * wx).astype(np.float32)
    c11 = (wy * wx).astype(np.float32)

    dmat = np.zeros((128, 2 * chunk), dtype=np.float32)
    for m in range(chunk):
        dmat[m, m] = 1.0            # I
        dmat[m + 1, chunk + m] = 1.0  # S

    cfg = Cfg(bpc=bpc, n=n, wpad=wpad, chunk=chunk)
    nbig = cfg.nbig
    rem = cfg.rem
    P = chunk + 1
    PS = (rem[1] + 1) if rem else 1

    in_maps = []
    for cidx in range(n_cores):
        sl = slice(cidx * bpc, (cidx + 1) * bpc)
        A = (ay[sl] + m_lo).astype(np.int64)
        Bc = (ax[sl] + m_lo).astype(np.int64)
        base = np.arange(bpc, dtype=np.int64) * (wpad * wpad)
        # flat element offsets: big windowed DMA start, remainder-chunk start
        off_big = base + A * wpad + Bc
        if rem:
            off_small = off_big + (rem[0]) * wpad
        else:
            off_small = np.zeros_like(off_big)
        offs = np.empty((1, bpc * 2), dtype=np.int32)
        offs[0, 0::2] = off_big
        offs[0, 1::2] = off_small
        wmat = np.empty((bpc, 128, 4), dtype=np.float32)
        wmat[:, :, 0] = c00[sl][:, None]
        wmat[:, :, 1] = c10[sl][:, None]
        wmat[:, :, 2] = c01[sl][:, None]
        wmat[:, :, 3] = c11[sl][:, None]
        in_maps.append({
            "x": np.ascontiguousarray(pp[sl]),
            "offs": offs,
            "wmat": wmat,
            "dmat": dmat,
        })
    return cfg, in_maps


N_CORES = 8
CHUNK = 127
_nc_cache: dict = {}


def kernel(padded_obj: np.ndarray, positions: np.ndarray) -> np.ndarray:
    padded_obj = np.asarray(padded_obj)
    positions = np.asarray(positions)
    B, npad, _, C = padded_obj.shape
    cfg, in_maps = host_prep(
        padded_obj.reshape(B, npad, npad).astype(np.float32, copy=False),
        positions, N_CORES, CHUNK)

    nc = _nc_cache.get(cfg)
    if nc is None:
        nc = build_nc(cfg)
        _nc_cache[cfg] = nc

    res = run_bass_kernel_spmd(nc, in_maps, core_ids=list(range(N_CORES)))
    out = np.concatenate([r["y"] for r in res.results], axis=0)
    return out.reshape(B, cfg.n, cfg.n, 1).astype(np.float32, copy=False)



# revision 7
# speedup vs baseline: 1.4578x; 1.0884x over previous
"""Trainium2 Bass kernel for nn_ExtractPatchesPositionLayer.

Reference semantics: per image b, bilinear-translate the (522,522,1) padded
object by t = -positions[b] (tfa.translate: out(y,x) = img(y+py, x+px),
zero fill outside), then center-crop 5px -> (512,512,1).

Because the shift is constant per image, floor/frac of the offset give an
integer window start (A,B) into the (zero-margin-padded) image plus four
constant bilinear corner weights. The whole bilinear then collapses into two
accumulating PE matmuls per 127-row chunk:

    psum[m, j] = sum_k Bv0[k, m] * W[k, j] + sum_k Bv1[k, m] * W[k, j+1]

with banded 128x127 matrices
    Bv0 = c00*I + c10*S,  Bv1 = c01*I + c11*S
    (I[k,m] = d_{k,m}, S[k,m] = d_{k,m+1};
     c00=(1-wy)(1-wx), c10=wy(1-wx), c01=(1-wy)wx, c11=wy*wx)

The per-image window is fetched with dynamic HWDGE DMAs: host-precomputed
flat element offsets (int32 data) are reg_load-ed into a small pool of
rotating SP registers and used as runtime AP offsets, so one SPMD program
serves all cores with no data-dependent immediates. (Indirect/gather DMA was
tried first but SWDGE lands every gather descriptor on DMA engine 0 —
1.4 ms serialized; dynamic HWDGE DMAs split across all 16 engines.)
Sharding: batch 256 -> 32 images x 8 cores, embarrassingly parallel,
no communication.
"""

from dataclasses import dataclass

import numpy as np

import concourse.bacc as bacc
import concourse.bass as bass
import concourse.mybir as mybir
import concourse.tile as tile
from concourse.bass_utils import run_bass_kernel_spmd


@dataclass(frozen=True)
class Cfg:
    bpc: int      # images per core
    n: int        # output height/width
    wpad: int     # padded input height/width (with zero margin)
    chunk: int    # output rows per matmul chunk (<=127)

    @property
    def win(self):  # window width loaded per chunk
        return self.n + 1

    @property
    def chunks(self):
        out = []
        r = 0
        while r < self.n:
            nr = min(self.chunk, self.n - r)
            out.append((r, nr))
            r += nr
        return out

    @property
    def nbig(self):
        return sum(1 for _, nr in self.chunks if nr == self.chunk)

    @property
    def rem(self):  # (row0, nrows) of the non-uniform trailing chunk, if any
        r = self.chunks[self.nbig:]
        assert len(r) <= 1
        return r[0] if r else None


def build_nc(cfg: Cfg) -> bass.Bass:
    BPC, N, WPAD, WIN = cfg.bpc, cfg.n, cfg.wpad, cfg.win
    CH = cfg.chunk
    nbig = cfg.nbig
    rem = cfg.rem
    P = CH + 1
    PS = (rem[1] + 1) if rem else 1  # partitions of the remainder gather
    TOT = BPC * WPAD * WPAD
    f32 = mybir.dt.float32
    f32r = mybir.dt.float32r  # PE fp32-reduced: 1 cycle/row (vs 4) at N>=256
    i32 = mybir.dt.int32

    nc = bacc.Bacc("TRN2", target_bir_lowering=False, debug=False)
    x_d = nc.declare_dram_parameter("x", [BPC, WPAD, WPAD], f32r, isOutput=False)
    offs_d = nc.declare_dram_parameter("offs", [1, BPC * 2], i32, isOutput=False)
    wmat_d = nc.declare_dram_parameter("wmat", [BPC, 128, 4], f32, isOutput=False)
    dmat_d = nc.declare_dram_parameter("dmat", [128, 2 * CH], f32, isOutput=False)
    y_d = nc.declare_dram_parameter("y", [BPC, N, N], f32, isOutput=True)

    with tile.TileContext(nc) as tc:
        with (
            tc.tile_pool(name="const", bufs=1) as constp,
            tc.tile_pool(name="bmat", bufs=3) as bmatp,
            tc.tile_pool(name="win", bufs=3) as winp,
            tc.tile_pool(name="outp", bufs=3) as outp,
            tc.tile_pool(name="ps", bufs=6, space="PSUM") as psp,
        ):
            dmat_sb = constp.tile([128, 2 * CH], f32, tag="dmat")
            nc.sync.dma_start(dmat_sb[:], dmat_d[:, :])
            wmat_sb = constp.tile([128, BPC * 4], f32, tag="wmat")
            nc.sync.dma_start(
                wmat_sb[:].rearrange("p (i q) -> p i q", q=4),
                wmat_d[:, :, :].transpose([1, 0, 2]),
            )
            offs_sb = constp.tile([1, BPC * 2], i32, tag="offs")
            nc.sync.dma_start(offs_sb[:], offs_d[:, :])
            d0 = dmat_sb[:, 0:CH]
            d1 = dmat_sb[:, CH:2 * CH]

            # two register pools, one per HWDGE ring (SP + ACT); alternating
            # the big window loads across both rings doubles descriptor-gen
            # fan-out (a single dynamic DMA's descriptors serialize on one
            # DMA engine otherwise)
            off_max = TOT - 1
            pools = []
            for eng_t, eng in ((mybir.EngineType.SP, nc.sync),
                               (mybir.EngineType.Activation, nc.scalar)):
                regs = [nc.alloc_register(eng_t, f"dynoff_{eng_t}_{k}")
                        for k in range(min(8, 2 * BPC))]
                svs = [nc.snap(r, donate=True, min_val=0, max_val=off_max)
                       for r in regs]
                pools.append((eng, regs, svs))

            for i in range(BPC):
                # per-image banded matrices Bv0, Bv1 on DVE
                b0 = bmatp.tile([128, CH], f32r, tag="b0")
                b1 = bmatp.tile([128, CH], f32r, tag="b1")
                t0 = bmatp.tile([128, CH], f32r, tag="t0")
                t1 = bmatp.tile([128, CH], f32r, tag="t1")
                c00 = wmat_sb[:, 4 * i + 0: 4 * i + 1]
                c10 = wmat_sb[:, 4 * i + 1: 4 * i + 2]
                c01 = wmat_sb[:, 4 * i + 2: 4 * i + 3]
                c11 = wmat_sb[:, 4 * i + 3: 4 * i + 4]
                nc.scalar.mul(t0[:], d1, c10)
                nc.scalar.mul(b0[:], d0, c00)
                nc.vector.tensor_add(b0[:], b0[:], t0[:])
                nc.scalar.mul(t1[:], d1, c11)
                nc.scalar.mul(b1[:], d0, c01)
                nc.vector.tensor_add(b1[:], b1[:], t1[:])

                # dynamic flat element offsets, host-precomputed per DMA;
                # one strided DMA loads all uniform chunks:
                # wt_big[p, c, w] = x.flat[off_big + (c*CH + p)*WPAD + w]
                eng, regs, svs = pools[i % 2]
                nreg = len(regs)
                kb = (2 * i) % nreg
                eng.reg_load(regs[kb], offs_sb[0:1, 2 * i: 2 * i + 1])
                wt_big = winp.tile([P, nbig * WIN], f32r, tag="wt_big")
                eng.dma_start(
                    wt_big[:].rearrange("p (c w) -> p c w", w=WIN),
                    bass.AP(x_d, svs[kb],
                            [[WPAD, P], [CH * WPAD, nbig], [1, WIN]]),
                )
                if rem:
                    ks = (2 * i + 1) % nreg
                    eng.reg_load(regs[ks], offs_sb[0:1, 2 * i + 1: 2 * i + 2])
                    wt_s = winp.tile([PS, WIN], f32r, tag="wt_s")
                    eng.dma_start(
                        wt_s[:],
                        bass.AP(x_d, svs[ks], [[WPAD, PS], [1, WIN]]),
                    )

                ob_big = outp.tile([CH, nbig * N], f32, tag="ob_big")
                for c in range(nbig):
                    ps = psp.tile([CH, N], f32, tag="ps")
                    rhs0 = wt_big[:P, c * WIN: c * WIN + N]
                    rhs1 = wt_big[:P, c * WIN + 1: c * WIN + 1 + N]
                    nc.tensor.matmul(out=ps[:], lhsT=b0[:P, :],
                                     rhs=rhs0,
                                     start=True, stop=False)
                    nc.tensor.matmul(out=ps[:], lhsT=b1[:P, :],
                                     rhs=rhs1,
                                     start=False, stop=True)
                    nc.scalar.copy(ob_big[:, c * N:(c + 1) * N], ps[:])
                # store the uniform chunks with one strided DMA. SWDGE
                # (gpsimd): HWDGE sends every SBUF->HBM descriptor to SDMA
                # engine 0 (trace: eng64 = 1.4ms busy, the critical path);
                # SWDGE's CounterMachine spreads them over all 16 engines.
                nc.gpsimd.dma_start(
                    bass.AP(y_d, i * (N * N),
                            [[N, CH], [CH * N, nbig], [1, N]]),
                    ob_big[:].rearrange("p (c w) -> p c w", w=N),
                )
                if rem:
                    r0r, nrr = rem
                    ps_s = psp.tile([CH, N], f32, tag="ps")
                    ob_s = outp.tile([max(nrr, 1), N], f32, tag="ob_s")
                    nc.tensor.matmul(out=ps_s[:nrr, :],
                                     lhsT=b0[:nrr + 1, :nrr],
                                     rhs=wt_s[:nrr + 1, 0:N],
                                     start=True, stop=False)
                    nc.tensor.matmul(out=ps_s[:nrr, :],
                                     lhsT=b1[:nrr + 1, :nrr],
                                     rhs=wt_s[:nrr + 1, 1:N + 1],
                                     start=False, stop=True)
                    nc.scalar.copy(ob_s[:nrr, :], ps_s[:nrr, :])
                    nc.gpsimd.dma_start(y_d[i, r0r:r0r + nrr, :], ob_s[:nrr, :])
    nc.compile()
    return nc


def host_prep(padded: np.ndarray, positions: np.ndarray, n_cores: int, chunk: int):
    """Shard + build metadata. padded: (B, npad, npad) f32, positions: (B, 2)."""
    B, npad, _ = padded.shape
    n = npad - 10
    bpc = B // n_cores
    win = n + 1

    px = positions[:, 0].astype(np.float32)
    py = positions[:, 1].astype(np.float32)
    fy = np.floor(py)
    fx = np.floor(px)
    ay = (5 + fy).astype(np.int64)
    ax = (5 + fx).astype(np.int64)
    wy = (py - fy).astype(np.float32)
    wx = (px - fx).astype(np.float32)

    m_lo = int(max(0, -min(ay.min(), ax.min())))
    m_hi = int(max(0, max(ay.max(), ax.max()) + win - npad))
    wpad = npad + m_lo + m_hi

    pp = np.zeros((B, wpad, wpad), dtype=np.float32)
    pp[:, m_lo:m_lo + npad, m_lo:m_lo + npad] = padded

    c00 = ((1 - wy) * (1 - wx)).astype(np.float32)
    c10 = (wy * (1 - wx)).astype(np.float32)
    c01 = ((1 - wy) * wx).astype(np.float32)
    c11 = (wy * wx).astype(np.float32)

    dmat = np.zeros((128, 2 * chunk), dtype=np.float32)
    for m in range(chunk):
        dmat[m, m] = 1.0            # I
        dmat[m + 1, chunk + m] = 1.0  # S

    cfg = Cfg(bpc=bpc, n=n, wpad=wpad, chunk=chunk)
    nbig = cfg.nbig
    rem = cfg.rem
    P = chunk + 1
    PS = (rem[1] + 1) if rem else 1

    in_maps = []
    for cidx in range(n_cores):
        sl = slice(cidx * bpc, (cidx + 1) * bpc)
        A = (ay[sl] + m_lo).astype(np.int64)
        Bc = (ax[sl] + m_lo).astype(np.int64)
        base = np.arange(bpc, dtype=np.int64) * (wpad * wpad)
        # flat element offsets: big windowed DMA start, remainder-chunk start
        off_big = base + A * wpad + Bc
        if rem:
            off_small = off_big + (rem[0]) * wpad
        else:
            off_small = np.zeros_like(off_big)
        offs = np.empty((1, bpc * 2), dtype=np.int32)
        offs[0, 0::2] = off_big
        offs[0, 1::2] = off_small
        wmat = np.empty((bpc, 128, 4), dtype=np.float32)
        wmat[:, :, 0] = c00[sl][:, None]
        wmat[:, :, 1] = c10[sl][:, None]
        wmat[:, :, 2] = c01[sl][:, None]
        wmat[:, :, 3] = c11[sl][:, None]
        in_maps.append({
            "x": np.ascontiguousarray(pp[sl]),
            "offs": offs,
            "wmat": wmat,
            "dmat": dmat,
        })
    return cfg, in_maps


N_CORES = 8
CHUNK = 127
_nc_cache: dict = {}


def kernel(padded_obj: np.ndarray, positions: np.ndarray) -> np.ndarray:
    padded_obj = np.asarray(padded_obj)
    positions = np.asarray(positions)
    B, npad, _, C = padded_obj.shape
    cfg, in_maps = host_prep(
        padded_obj.reshape(B, npad, npad).astype(np.float32, copy=False),
        positions, N_CORES, CHUNK)

    nc = _nc_cache.get(cfg)
    if nc is None:
        nc = build_nc(cfg)
        _nc_cache[cfg] = nc

    res = run_bass_kernel_spmd(nc, in_maps, core_ids=list(range(N_CORES)))
    out = np.concatenate([r["y"] for r in res.results], axis=0)
    return out.reshape(B, cfg.n, cfg.n, 1).astype(np.float32, copy=False)



# revision 8
# speedup vs baseline: 1.4978x; 1.0275x over previous
"""Trainium2 Bass kernel for nn_ExtractPatchesPositionLayer.

Reference semantics: per image b, bilinear-translate the (522,522,1) padded
object by t = -positions[b] (tfa.translate: out(y,x) = img(y+py, x+px),
zero fill outside), then center-crop 5px -> (512,512,1).

Because the shift is constant per image, floor/frac of the offset give an
integer window start (A,B) into the (zero-margin-padded) image plus four
constant bilinear corner weights. The whole bilinear then collapses into two
accumulating PE matmuls per 127-row chunk:

    psum[m, j] = sum_k Bv0[k, m] * W[k, j] + sum_k Bv1[k, m] * W[k, j+1]

with banded 128x127 matrices
    Bv0 = c00*I + c10*S,  Bv1 = c01*I + c11*S
    (I[k,m] = d_{k,m}, S[k,m] = d_{k,m+1};
     c00=(1-wy)(1-wx), c10=wy(1-wx), c01=(1-wy)wx, c11=wy*wx)

The per-image window is fetched with dynamic HWDGE DMAs: host-precomputed
flat element offsets (int32 data) are reg_load-ed into a small pool of
rotating SP registers and used as runtime AP offsets, so one SPMD program
serves all cores with no data-dependent immediates. (Indirect/gather DMA was
tried first but SWDGE lands every gather descriptor on DMA engine 0 —
1.4 ms serialized; dynamic HWDGE DMAs split across all 16 engines.)
Sharding: batch 256 -> 32 images x 8 cores, embarrassingly parallel,
no communication.
"""

from dataclasses import dataclass

import numpy as np

import concourse.bacc as bacc
import concourse.bass as bass
import concourse.mybir as mybir
import concourse.tile as tile
from concourse.bass_utils import run_bass_kernel_spmd


@dataclass(frozen=True)
class Cfg:
    bpc: int      # images per core
    n: int        # output height/width
    wpad: int     # padded input height/width (with zero margin)
    chunk: int    # output rows per matmul chunk (<=127)

    @property
    def win(self):  # window width loaded per chunk
        return self.n + 1

    @property
    def chunks(self):
        out = []
        r = 0
        while r < self.n:
            nr = min(self.chunk, self.n - r)
            out.append((r, nr))
            r += nr
        return out

    @property
    def nbig(self):
        return sum(1 for _, nr in self.chunks if nr == self.chunk)

    @property
    def rem(self):  # (row0, nrows) of the non-uniform trailing chunk, if any
        r = self.chunks[self.nbig:]
        assert len(r) <= 1
        return r[0] if r else None


def build_nc(cfg: Cfg) -> bass.Bass:
    BPC, N, WPAD, WIN = cfg.bpc, cfg.n, cfg.wpad, cfg.win
    CH = cfg.chunk
    nbig = cfg.nbig
    rem = cfg.rem
    P = CH + 1
    PS = (rem[1] + 1) if rem else 1  # partitions of the remainder gather
    TOT = BPC * WPAD * WPAD
    f32 = mybir.dt.float32
    f32r = mybir.dt.float32r  # PE fp32-reduced: 1 cycle/row (vs 4) at N>=256
    i32 = mybir.dt.int32

    nc = bacc.Bacc("TRN2", target_bir_lowering=False, debug=False)
    x_d = nc.declare_dram_parameter("x", [BPC, WPAD, WPAD], f32r, isOutput=False)
    offs_d = nc.declare_dram_parameter("offs", [1, BPC * 2], i32, isOutput=False)
    wmat_d = nc.declare_dram_parameter("wmat", [BPC, 128, 4], f32, isOutput=False)
    dmat_d = nc.declare_dram_parameter("dmat", [128, 2 * CH], f32, isOutput=False)
    y_d = nc.declare_dram_parameter("y", [BPC, N, N], f32, isOutput=True)

    with tile.TileContext(nc) as tc:
        with (
            tc.tile_pool(name="const", bufs=1) as constp,
            tc.tile_pool(name="bmat", bufs=6) as bmatp,
            tc.tile_pool(name="win", bufs=6) as winp,
            tc.tile_pool(name="outp", bufs=6) as outp,
            tc.tile_pool(name="ps", bufs=8, space="PSUM") as psp,
        ):
            dmat_sb = constp.tile([128, 2 * CH], f32, tag="dmat")
            nc.sync.dma_start(dmat_sb[:], dmat_d[:, :])
            wmat_sb = constp.tile([128, BPC * 4], f32, tag="wmat")
            nc.sync.dma_start(
                wmat_sb[:].rearrange("p (i q) -> p i q", q=4),
                wmat_d[:, :, :].transpose([1, 0, 2]),
            )
            offs_sb = constp.tile([1, BPC * 2], i32, tag="offs")
            nc.sync.dma_start(offs_sb[:], offs_d[:, :])
            d0 = dmat_sb[:, 0:CH]
            d1 = dmat_sb[:, CH:2 * CH]

            # two register pools, one per HWDGE ring (SP + ACT); alternating
            # the big window loads across both rings doubles descriptor-gen
            # fan-out (a single dynamic DMA's descriptors serialize on one
            # DMA engine otherwise)
            off_max = TOT - 1
            pools = []
            for eng_t, eng in ((mybir.EngineType.SP, nc.sync),
                               (mybir.EngineType.Activation, nc.scalar)):
                regs = [nc.alloc_register(eng_t, f"dynoff_{eng_t}_{k}")
                        for k in range(min(16, 2 * BPC))]
                svs = [nc.snap(r, donate=True, min_val=0, max_val=off_max)
                       for r in regs]
                pools.append((eng, regs, svs))

            for i in range(BPC):
                # per-image banded matrices Bv0, Bv1 on DVE
                b0 = bmatp.tile([128, CH], f32r, tag="b0")
                b1 = bmatp.tile([128, CH], f32r, tag="b1")
                t0 = bmatp.tile([128, CH], f32r, tag="t0")
                t1 = bmatp.tile([128, CH], f32r, tag="t1")
                c00 = wmat_sb[:, 4 * i + 0: 4 * i + 1]
                c10 = wmat_sb[:, 4 * i + 1: 4 * i + 2]
                c01 = wmat_sb[:, 4 * i + 2: 4 * i + 3]
                c11 = wmat_sb[:, 4 * i + 3: 4 * i + 4]
                nc.scalar.mul(t0[:], d1, c10)
                nc.scalar.mul(b0[:], d0, c00)
                nc.vector.tensor_add(b0[:], b0[:], t0[:])
                nc.scalar.mul(t1[:], d1, c11)
                nc.scalar.mul(b1[:], d0, c01)
                nc.vector.tensor_add(b1[:], b1[:], t1[:])

                # dynamic flat element offsets, host-precomputed per DMA;
                # one strided DMA loads all uniform chunks:
                # wt_big[p, c, w] = x.flat[off_big + (c*CH + p)*WPAD + w]
                eng, regs, svs = pools[i % 2]
                nreg = len(regs)
                kb = (2 * i) % nreg
                eng.reg_load(regs[kb], offs_sb[0:1, 2 * i: 2 * i + 1])
                wt_big = winp.tile([P, nbig * WIN], f32r, tag="wt_big")
                eng.dma_start(
                    wt_big[:].rearrange("p (c w) -> p c w", w=WIN),
                    bass.AP(x_d, svs[kb],
                            [[WPAD, P], [CH * WPAD, nbig], [1, WIN]]),
                )
                if rem:
                    ks = (2 * i + 1) % nreg
                    eng.reg_load(regs[ks], offs_sb[0:1, 2 * i + 1: 2 * i + 2])
                    wt_s = winp.tile([PS, WIN], f32r, tag="wt_s")
                    eng.dma_start(
                        wt_s[:],
                        bass.AP(x_d, svs[ks], [[WPAD, PS], [1, WIN]]),
                    )

                ob_big = outp.tile([CH, nbig * N], f32, tag="ob_big")
                for c in range(nbig):
                    ps = psp.tile([CH, N], f32, tag="ps")
                    rhs0 = wt_big[:P, c * WIN: c * WIN + N]
                    rhs1 = wt_big[:P, c * WIN + 1: c * WIN + 1 + N]
                    nc.tensor.matmul(out=ps[:], lhsT=b0[:P, :],
                                     rhs=rhs0,
                                     start=True, stop=False)
                    nc.tensor.matmul(out=ps[:], lhsT=b1[:P, :],
                                     rhs=rhs1,
                                     start=False, stop=True)
                    nc.vector.tensor_copy(ob_big[:, c * N:(c + 1) * N], ps[:])
                # store the uniform chunks with one strided DMA. SWDGE
                # (gpsimd): HWDGE sends every SBUF->HBM descriptor to SDMA
                # engine 0 (trace: eng64 = 1.4ms busy, the critical path);
                # SWDGE's CounterMachine spreads them over all 16 engines.
                nc.gpsimd.dma_start(
                    bass.AP(y_d, i * (N * N),
                            [[N, CH], [CH * N, nbig], [1, N]]),
                    ob_big[:].rearrange("p (c w) -> p c w", w=N),
                )
                if rem:
                    r0r, nrr = rem
                    ps_s = psp.tile([CH, N], f32, tag="ps")
                    ob_s = outp.tile([max(nrr, 1), N], f32, tag="ob_s")
                    nc.tensor.matmul(out=ps_s[:nrr, :],
                                     lhsT=b0[:nrr + 1, :nrr],
                                     rhs=wt_s[:nrr + 1, 0:N],
                                     start=True, stop=False)
                    nc.tensor.matmul(out=ps_s[:nrr, :],
                                     lhsT=b1[:nrr + 1, :nrr],
                                     rhs=wt_s[:nrr + 1, 1:N + 1],
                                     start=False, stop=True)
                    nc.vector.tensor_copy(ob_s[:nrr, :], ps_s[:nrr, :])
                    nc.gpsimd.dma_start(y_d[i, r0r:r0r + nrr, :], ob_s[:nrr, :])
    nc.compile()
    return nc


def host_prep(padded: np.ndarray, positions: np.ndarray, n_cores: int, chunk: int):
    """Shard + build metadata. padded: (B, npad, npad) f32, positions: (B, 2)."""
    B, npad, _ = padded.shape
    n = npad - 10
    bpc = B // n_cores
    win = n + 1

    px = positions[:, 0].astype(np.float32)
    py = positions[:, 1].astype(np.float32)
    fy = np.floor(py)
    fx = np.floor(px)
    ay = (5 + fy).astype(np.int64)
    ax = (5 + fx).astype(np.int64)
    wy = (py - fy).astype(np.float32)
    wx = (px - fx).astype(np.float32)

    m_lo = int(max(0, -min(ay.min(), ax.min())))
    m_hi = int(max(0, max(ay.max(), ax.max()) + win - npad))
    wpad = npad + m_lo + m_hi

    pp = np.zeros((B, wpad, wpad), dtype=np.float32)
    pp[:, m_lo:m_lo + npad, m_lo:m_lo + npad] = padded

    c00 = ((1 - wy) * (1 - wx)).astype(np.float32)
    c10 = (wy * (1 - wx)).astype(np.float32)
    c01 = ((1 - wy) * wx).astype(np.float32)
    c11 = (wy * wx).astype(np.float32)

    dmat = np.zeros((128, 2 * chunk), dtype=np.float32)
    for m in range(chunk):
        dmat[m, m] = 1.0            # I
        dmat[m + 1, chunk + m] = 1.0  # S

    cfg = Cfg(bpc=bpc, n=n, wpad=wpad, chunk=chunk)
    nbig = cfg.nbig
    rem = cfg.rem
    P = chunk + 1
    PS = (rem[1] + 1) if rem else 1

    in_maps = []
    for cidx in range(n_cores):
        sl = slice(cidx * bpc, (cidx + 1) * bpc)
        A = (ay[sl] + m_lo).astype(np.int64)
        Bc = (ax[sl] + m_lo).astype(np.int64)
        base = np.arange(bpc, dtype=np.int64) * (wpad * wpad)
        # flat element offsets: big windowed DMA start, remainder-chunk start
        off_big = base + A * wpad + Bc
        if rem:
            off_small = off_big + (rem[0]) * wpad
        else:
            off_small = np.zeros_like(off_big)
        offs = np.empty((1, bpc * 2), dtype=np.int32)
        offs[0, 0::2] = off_big
        offs[0, 1::2] = off_small
        wmat = np.empty((bpc, 128, 4), dtype=np.float32)
        wmat[:, :, 0] = c00[sl][:, None]
        wmat[:, :, 1] = c10[sl][:, None]
        wmat[:, :, 2] = c01[sl][:, None]
        wmat[:, :, 3] = c11[sl][:, None]
        in_maps.append({
            "x": np.ascontiguousarray(pp[sl]),
            "offs": offs,
            "wmat": wmat,
            "dmat": dmat,
        })
    return cfg, in_maps


N_CORES = 8
CHUNK = 127
_nc_cache: dict = {}


def kernel(padded_obj: np.ndarray, positions: np.ndarray) -> np.ndarray:
    padded_obj = np.asarray(padded_obj)
    positions = np.asarray(positions)
    B, npad, _, C = padded_obj.shape
    cfg, in_maps = host_prep(
        padded_obj.reshape(B, npad, npad).astype(np.float32, copy=False),
        positions, N_CORES, CHUNK)

    nc = _nc_cache.get(cfg)
    if nc is None:
        nc = build_nc(cfg)
        _nc_cache[cfg] = nc

    res = run_bass_kernel_spmd(nc, in_maps, core_ids=list(range(N_CORES)))
    out = np.concatenate([r["y"] for r in res.results], axis=0)
    return out.reshape(B, cfg.n, cfg.n, 1).astype(np.float32, copy=False)



# revision 9
# speedup vs baseline: 4.5539x; 3.0404x over previous
"""Trainium2 Bass kernel for nn_ExtractPatchesPositionLayer.

Reference semantics: per image b, bilinear-translate the (522,522,1) padded
object by t = -positions[b] (tfa.translate: out(y,x) = img(y+py, x+px),
zero fill outside), then center-crop 5px -> (512,512,1).

Because the shift is constant per image, floor/frac of the offset give an
integer window start (A,B) into the (zero-margin-padded) image plus four
constant bilinear corner weights:

    out[r, j] = c00*W[r, j] + c01*W[r, j+1] + c10*W[r+1, j] + c11*W[r+1, j+1]
    W[r, c] = pp[A+r, B+c]

Layout trick: SBUF partition p holds FIVE consecutive padded-image rows
(A+4p .. A+4p+4) as ONE contiguous DRAM span (4*wpad+513 elements, a single
~10.5 KB line-rate DMA descriptor per partition).  Output rows 4p..4p+3 then
depend only on partition p, so the whole bilinear is four fused
multiply-accumulate passes with free-dim shifts (dy*wpad + dx) -- no PE, no
PSUM, no cross-partition shuffles, no remainder chunk.  The output tile holds
4 consecutive y rows per partition = one contiguous 8 KB descriptor each.

DMA routing (hard-won trace facts):
  * inputs: dynamic HWDGE on the SP ring (runtime reg offsets; descriptors
    spread over all 16 SDMA engines by dest SBUF partition).
  * outputs: SWDGE via gpsimd -- HWDGE sends every SBUF->HBM descriptor to
    SDMA engine 0 (1.4 ms serialized); SWDGE's CounterMachine spreads them.
Sharding: batch 256 -> 32 images x 8 cores, embarrassingly parallel.
"""

from dataclasses import dataclass

import numpy as np

import concourse.bacc as bacc
import concourse.bass as bass
import concourse.mybir as mybir
import concourse.tile as tile
from concourse.bass_utils import run_bass_kernel_spmd


@dataclass(frozen=True)
class Cfg:
    bpc: int      # images per core
    n: int        # output height/width
    wpad: int     # padded input height/width (with zero margin)
    xlen: int     # flat padded-input length per core (incl. tail pad)

    @property
    def wrow(self):  # output rows per partition
        return self.n // 128

    @property
    def span(self):  # elements DMA'd per partition
        return self.wrow * self.wpad + self.n + 1


def build_nc(cfg: Cfg) -> bass.Bass:
    BPC, N, WPAD = cfg.bpc, cfg.n, cfg.wpad
    WR = cfg.wrow
    SPAN = cfg.span
    # tile is over-allocated so the strided (dy=1) views can be *constructed*
    # by slice+rearrange; the actual element APs stay within SPAN
    SPAN_A = (WR + 1) * WPAD + 2
    XLEN = cfg.xlen
    f32 = mybir.dt.float32
    i32 = mybir.dt.int32
    MUL = mybir.AluOpType.mult
    ADD = mybir.AluOpType.add

    nc = bacc.Bacc("TRN2", target_bir_lowering=False, debug=False)
    x_d = nc.declare_dram_parameter("x", [1, XLEN], f32, isOutput=False)
    offs_d = nc.declare_dram_parameter("offs", [1, BPC], i32, isOutput=False)
    wmat_d = nc.declare_dram_parameter("wmat", [BPC, 128, 4], f32, isOutput=False)
    y_d = nc.declare_dram_parameter("y", [BPC, N, N], f32, isOutput=True)

    with tile.TileContext(nc) as tc:
        with (
            tc.tile_pool(name="const", bufs=1) as constp,
            tc.tile_pool(name="win", bufs=4) as winp,
            tc.tile_pool(name="hp", bufs=4) as hp,
            tc.tile_pool(name="op", bufs=4) as op,
        ):
            wmat_sb = constp.tile([128, BPC * 4], f32, tag="wmat")
            nc.sync.dma_start(
                wmat_sb[:].rearrange("p (i q) -> p i q", q=4),
                wmat_d[:, :, :].transpose([1, 0, 2]),
            )
            offs_sb = constp.tile([1, BPC], i32, tag="offs")
            nc.sync.dma_start(offs_sb[:], offs_d[:, :])

            regs = [nc.alloc_register(mybir.EngineType.SP, f"dynoff_{k}")
                    for k in range(min(16, BPC))]
            svs = [nc.snap(r, donate=True, min_val=0, max_val=XLEN - 1)
                   for r in regs]
            nreg = len(regs)

            for i in range(BPC):
                k = i % nreg
                nc.sync.reg_load(regs[k], offs_sb[0:1, i: i + 1])
                wt = winp.tile([128, SPAN_A], f32, tag="wt")
                nc.sync.dma_start(
                    wt[:, 0:SPAN],
                    bass.AP(x_d, svs[k], [[WR * WPAD, 128], [1, SPAN]]),
                )

                def W(dy, dx):
                    a = dy * WPAD + dx
                    return wt[:, a: a + WR * WPAD].rearrange(
                        "p (u j) -> p u j", j=WPAD)[:, :, 0:N]

                c00 = wmat_sb[:, 4 * i + 0: 4 * i + 1]
                c10 = wmat_sb[:, 4 * i + 1: 4 * i + 2]
                c01 = wmat_sb[:, 4 * i + 2: 4 * i + 3]
                c11 = wmat_sb[:, 4 * i + 3: 4 * i + 4]

                h0 = hp.tile([128, WR * N], f32, tag="h0")
                ob = op.tile([128, WR * N], f32, tag="ob")
                h0v = h0[:].rearrange("p (u j) -> p u j", j=N)
                obv = ob[:].rearrange("p (u j) -> p u j", j=N)

                nc.scalar.mul(h0v, W(0, 0), c00)
                nc.vector.scalar_tensor_tensor(h0v, W(0, 1), c01, h0v, MUL, ADD)
                nc.scalar.mul(obv, W(1, 0), c10)
                nc.vector.scalar_tensor_tensor(obv, W(1, 1), c11, obv, MUL, ADD)
                nc.gpsimd.tensor_add(ob[:], ob[:], h0[:])

                nc.gpsimd.dma_start(
                    bass.AP(y_d, i * (N * N), [[WR * N, 128], [1, WR * N]]),
                    ob[:],
                )
    nc.compile()
    return nc


def host_prep(padded: np.ndarray, positions: np.ndarray, n_cores: int):
    """Shard + build metadata. padded: (B, npad, npad) f32, positions: (B, 2)."""
    B, npad, _ = padded.shape
    n = npad - 10
    bpc = B // n_cores

    px = positions[:, 0].astype(np.float32)
    py = positions[:, 1].astype(np.float32)
    fy = np.floor(py)
    fx = np.floor(px)
    ay = (5 + fy).astype(np.int64)
    ax = (5 + fx).astype(np.int64)
    wy = (py - fy).astype(np.float32)
    wx = (px - fx).astype(np.float32)

    m_lo = int(max(0, -min(ay.min(), ax.min())))
    m_hi = int(max(0, max(ay.max(), ax.max()) + n + 1 - npad))
    wpad = npad + m_lo + m_hi

    pp = np.zeros((B, wpad, wpad), dtype=np.float32)
    pp[:, m_lo:m_lo + npad, m_lo:m_lo + npad] = padded

    c00 = ((1 - wy) * (1 - wx)).astype(np.float32)
    c10 = (wy * (1 - wx)).astype(np.float32)
    c01 = ((1 - wy) * wx).astype(np.float32)
    c11 = (wy * wx).astype(np.float32)

    A = ay + m_lo
    Bc = ax + m_lo
    base = (np.arange(B, dtype=np.int64) % bpc) * (wpad * wpad)
    off = base + A * wpad + Bc

    wr = n // 128
    span = wr * wpad + n + 1
    # flat length incl. tail so the last image's strided span stays in bounds
    need = int(off.max()) + 127 * wr * wpad + span
    xlen = max(bpc * wpad * wpad, need)

    cfg = Cfg(bpc=bpc, n=n, wpad=wpad, xlen=xlen)

    in_maps = []
    for cidx in range(n_cores):
        sl = slice(cidx * bpc, (cidx + 1) * bpc)
        flat = np.zeros((1, xlen), dtype=np.float32)
        flat[0, :bpc * wpad * wpad] = pp[sl].reshape(-1)
        offs = off[sl].astype(np.int32).reshape(1, bpc)
        wmat = np.empty((bpc, 128, 4), dtype=np.float32)
        wmat[:, :, 0] = c00[sl][:, None]
        wmat[:, :, 1] = c10[sl][:, None]
        wmat[:, :, 2] = c01[sl][:, None]
        wmat[:, :, 3] = c11[sl][:, None]
        in_maps.append({"x": flat, "offs": offs, "wmat": wmat})
    return cfg, in_maps


N_CORES = 8
_nc_cache: dict = {}


def kernel(padded_obj: np.ndarray, positions: np.ndarray) -> np.ndarray:
    padded_obj = np.asarray(padded_obj)
    positions = np.asarray(positions)
    B, npad, _, C = padded_obj.shape
    cfg, in_maps = host_prep(
        padded_obj.reshape(B, npad, npad).astype(np.float32, copy=False),
        positions, N_CORES)

    nc = _nc_cache.get(cfg)
    if nc is None:
        nc = build_nc(cfg)
        _nc_cache[cfg] = nc

    res = run_bass_kernel_spmd(nc, in_maps, core_ids=list(range(N_CORES)))
    out = np.concatenate([r["y"] for r in res.results], axis=0)
    return out.reshape(B, cfg.n, cfg.n, 1).astype(np.float32, copy=False)


# revision 10
# speedup vs baseline: 5.6453x; 1.2397x over previous
"""Trainium2 Bass kernel for nn_ExtractPatchesPositionLayer.

Reference semantics: per image b, bilinear-translate the (522,522,1) padded
object by t = -positions[b] (tfa.translate: out(y,x) = img(y+py, x+px),
zero fill outside), then center-crop 5px -> (512,512,1).

Because the shift is constant per image, floor/frac of the offset give an
integer window start (A,B) into the (zero-margin-padded) image plus four
constant bilinear corner weights:

    out[r, j] = c00*W[r, j] + c01*W[r, j+1] + c10*W[r+1, j] + c11*W[r+1, j+1]
    W[r, c] = pp[A+r, B+c]

Layout trick: SBUF partition p holds FIVE consecutive padded-image rows
(A+4p .. A+4p+4) as ONE contiguous DRAM span (4*wpad+513 elements, a single
~10.5 KB line-rate DMA descriptor per partition).  Output rows 4p..4p+3 then
depend only on partition p, so the whole bilinear is four fused
multiply-accumulate passes with free-dim shifts (dy*wpad + dx) -- no PE, no
PSUM, no cross-partition shuffles, no remainder chunk.  The output tile holds
4 consecutive y rows per partition = one contiguous 8 KB descriptor each.

DMA routing (hard-won trace facts):
  * inputs: dynamic HWDGE on the SP ring (runtime reg offsets; descriptors
    spread over all 16 SDMA engines by dest SBUF partition).
  * outputs: SWDGE via gpsimd -- HWDGE sends every SBUF->HBM descriptor to
    SDMA engine 0 (1.4 ms serialized); SWDGE's CounterMachine spreads them.
Sharding: batch 256 -> 32 images x 8 cores, embarrassingly parallel.
"""

from dataclasses import dataclass

import numpy as np

import concourse.bacc as bacc
import concourse.bass as bass
import concourse.mybir as mybir
import concourse.tile as tile
from concourse.bass_utils import run_bass_kernel_spmd


@dataclass(frozen=True)
class Cfg:
    bpc: int      # images per core
    n: int        # output height/width
    wpad: int     # padded input height/width (with zero margin)
    xlen: int     # flat padded-input length per core (incl. tail pad)

    @property
    def wrow(self):  # output rows per partition
        return self.n // 128

    @property
    def span(self):  # elements DMA'd per partition (WR+1 full rows + 2)
        return (self.wrow + 1) * self.wpad + 2


def build_nc(cfg: Cfg) -> bass.Bass:
    BPC, N, WPAD = cfg.bpc, cfg.n, cfg.wpad
    WR = cfg.wrow
    SPAN = cfg.span
    WIDE = WR * WPAD  # full-width output row block per partition
    XLEN = cfg.xlen
    f32 = mybir.dt.float32
    i32 = mybir.dt.int32
    MUL = mybir.AluOpType.mult
    ADD = mybir.AluOpType.add

    nc = bacc.Bacc("TRN2", target_bir_lowering=False, debug=False)
    x_d = nc.declare_dram_parameter("x", [1, XLEN], f32, isOutput=False)
    offs_d = nc.declare_dram_parameter("offs", [1, BPC], i32, isOutput=False)
    wmat_d = nc.declare_dram_parameter("wmat", [BPC, 128, 4], f32, isOutput=False)
    y_d = nc.declare_dram_parameter("y", [BPC, N, WPAD], f32, isOutput=True)

    with tile.TileContext(nc) as tc:
        with (
            tc.tile_pool(name="const", bufs=1) as constp,
            tc.tile_pool(name="win", bufs=4) as winp,
            tc.tile_pool(name="hp", bufs=4) as hp,
            tc.tile_pool(name="op", bufs=4) as op,
        ):
            wmat_sb = constp.tile([128, BPC * 4], f32, tag="wmat")
            nc.sync.dma_start(
                wmat_sb[:].rearrange("p (i q) -> p i q", q=4),
                wmat_d[:, :, :].transpose([1, 0, 2]),
            )
            offs_sb = constp.tile([1, BPC], i32, tag="offs")
            nc.sync.dma_start(offs_sb[:], offs_d[:, :])

            regs = [nc.alloc_register(mybir.EngineType.SP, f"dynoff_{k}")
                    for k in range(min(16, BPC))]
            svs = [nc.snap(r, donate=True, min_val=0, max_val=XLEN - 1)
                   for r in regs]
            nreg = len(regs)

            for i in range(BPC):
                k = i % nreg
                nc.sync.reg_load(regs[k], offs_sb[0:1, i: i + 1])
                wt = winp.tile([128, SPAN], f32, tag="wt")
                nc.sync.dma_start(
                    wt[:],
                    bass.AP(x_d, svs[k], [[WR * WPAD, 128], [1, SPAN]]),
                )
                # full-width CONTIGUOUS views (junk between rows is computed
                # and later trimmed on host): DVE runs 2 elem/cycle on flat
                # APs vs 1 on strided 3D ones
                w00 = wt[:, 0: WIDE]
                w01 = wt[:, 1: WIDE + 1]
                w10 = wt[:, WPAD: WPAD + WIDE]
                w11 = wt[:, WPAD + 1: WPAD + WIDE + 1]

                c00 = wmat_sb[:, 4 * i + 0: 4 * i + 1]
                c10 = wmat_sb[:, 4 * i + 1: 4 * i + 2]
                c01 = wmat_sb[:, 4 * i + 2: 4 * i + 3]
                c11 = wmat_sb[:, 4 * i + 3: 4 * i + 4]

                h0 = hp.tile([128, WIDE], f32, tag="h0")
                ob = op.tile([128, WIDE], f32, tag="ob")

                nc.scalar.mul(h0[:], w00, c00)
                nc.vector.scalar_tensor_tensor(h0[:], w01, c01, h0[:], MUL, ADD)
                nc.scalar.mul(ob[:], w10, c10)
                nc.vector.scalar_tensor_tensor(ob[:], w11, c11, ob[:], MUL, ADD)
                nc.vector.tensor_add(ob[:], ob[:], h0[:])

                nc.gpsimd.dma_start(
                    bass.AP(y_d, i * (N * WPAD), [[WIDE, 128], [1, WIDE]]),
                    ob[:],
                )
    nc.compile()
    return nc


def host_prep(padded: np.ndarray, positions: np.ndarray, n_cores: int):
    """Shard + build metadata. padded: (B, npad, npad) f32, positions: (B, 2)."""
    B, npad, _ = padded.shape
    n = npad - 10
    bpc = B // n_cores

    px = positions[:, 0].astype(np.float32)
    py = positions[:, 1].astype(np.float32)
    fy = np.floor(py)
    fx = np.floor(px)
    ay = (5 + fy).astype(np.int64)
    ax = (5 + fx).astype(np.int64)
    wy = (py - fy).astype(np.float32)
    wx = (px - fx).astype(np.float32)

    m_lo = int(max(0, -min(ay.min(), ax.min())))
    m_hi = int(max(0, max(ay.max(), ax.max()) + n + 1 - npad))
    wpad = npad + m_lo + m_hi

    pp = np.zeros((B, wpad, wpad), dtype=np.float32)
    pp[:, m_lo:m_lo + npad, m_lo:m_lo + npad] = padded

    c00 = ((1 - wy) * (1 - wx)).astype(np.float32)
    c10 = (wy * (1 - wx)).astype(np.float32)
    c01 = ((1 - wy) * wx).astype(np.float32)
    c11 = (wy * wx).astype(np.float32)

    A = ay + m_lo
    Bc = ax + m_lo
    base = (np.arange(B, dtype=np.int64) % bpc) * (wpad * wpad)
    off = base + A * wpad + Bc

    wr = n // 128
    span = (wr + 1) * wpad + 2
    # flat length incl. tail so the last image's strided span stays in bounds
    need = int(off.max()) + 127 * wr * wpad + span
    xlen = max(bpc * wpad * wpad, need)

    cfg = Cfg(bpc=bpc, n=n, wpad=wpad, xlen=xlen)

    in_maps = []
    for cidx in range(n_cores):
        sl = slice(cidx * bpc, (cidx + 1) * bpc)
        flat = np.zeros((1, xlen), dtype=np.float32)
        flat[0, :bpc * wpad * wpad] = pp[sl].reshape(-1)
        offs = off[sl].astype(np.int32).reshape(1, bpc)
        wmat = np.empty((bpc, 128, 4), dtype=np.float32)
        wmat[:, :, 0] = c00[sl][:, None]
        wmat[:, :, 1] = c10[sl][:, None]
        wmat[:, :, 2] = c01[sl][:, None]
        wmat[:, :, 3] = c11[sl][:, None]
        in_maps.append({"x": flat, "offs": offs, "wmat": wmat})
    return cfg, in_maps


N_CORES = 8
_nc_cache: dict = {}


def kernel(padded_obj: np.ndarray, positions: np.ndarray) -> np.ndarray:
    padded_obj = np.asarray(padded_obj)
    positions = np.asarray(positions)
    B, npad, _, C = padded_obj.shape
    cfg, in_maps = host_prep(
        padded_obj.reshape(B, npad, npad).astype(np.float32, copy=False),
        positions, N_CORES)

    nc = _nc_cache.get(cfg)
    if nc is None:
        nc = build_nc(cfg)
        _nc_cache[cfg] = nc

    res = run_bass_kernel_spmd(nc, in_maps, core_ids=list(range(N_CORES)))
    out = np.concatenate([r["y"][:, :, :cfg.n] for r in res.results], axis=0)
    return np.ascontiguousarray(out).reshape(B, cfg.n, cfg.n, 1)
